# revision 41
# baseline (speedup 1.0000x reference)
"""Trainium2 Bass kernel for nn_AttentionDecoderModel (decoder layer:
self-attn + cross-attn + DoubleSwish FFN + BasicNorm + bypass).

Strategy: pure data-parallel over batch (16 batches / 8 cores = 2 per core),
no collectives. Activations live in transposed [feature, token] layout
on-chip; every matmul contracts over partitions with zero on-chip
transposes.

v2 (this file): fp8e4 + MatmulPerfMode.DoubleRow for every large matmul
whose contraction is >= 256 (Q/K/V projections, attn*V, out-proj, both FFN
matmuls): two 128-deep k-tiles are contracted per pass, halving PE column
count vs bf16. Scores stay bf16 (K=64 per head, no DR win, and exp
amplifies score error). Softmax: exp(score - 1.5) in fp8e4 (uniform scale
cancels in the normalization); denominator comes free from a ones-column
packed into V (row 32 of each head's 33-column slot); all 4 denominators
of a 4-head half are broadcast by ONE rank-4 matmul then inverted with one
DVE reciprocal_approx_fast. DoubleSwish h*sigmoid(h-1) = (h+b1) * 0.5 *
(tanh(0.5(h+b1)-0.5)+1) so the only ACT tables used anywhere are
exp/tanh/square/identity (all in `exp_and_others` -- zero table swaps;
BasicNorm's rsqrt is computed sqrt-free with the int32 bit-trick + 2
Newton steps on a [1,T] row, then broadcast by a rank-1 fp32r matmul).

Element-wise work is spread over all three vector-ish engines: ACT gets
the Q/V psum->sbuf copies (idle during projections) + exp/tanh, Pool
(gpsimd) gets all SBUF->SBUF masking/copy/square work, DVE the rest.

dtypes: fp8 operands are uint8 bit patterns of ml_dtypes.float8_e4m3
prepared on the host (free); residual x carried in bf16; all PSUM fp32.
Host-side verifies the masks match the causal/all-valid pattern the
schedule hardcodes (anything else falls back to a numpy reference).
"""
import numpy as np

B, T, S, D, A, NH = 16, 512, 1024, 512, 512, 8
HD, HD2, A2, FF = 64, 32, 256, 2048
NCORES, BPC = 8, 2
DT = D // 128          # 4 d-tiles
KP = 2                 # k-tile pairs over D

_RUNNER = None


# ----------------------------------------------------------------------------
# graph builder
# ----------------------------------------------------------------------------

def build_nc(unroll=1, taps=(), inline_data=None, stop_after=None):
    import concourse.bass as bass
    import concourse.tile as tile
    import concourse.mybir as mybir
    from concourse import bacc
    from contextlib import ExitStack

    f32 = mybir.dt.float32
    fr = mybir.dt.float32r
    bf = mybir.dt.bfloat16
    f8 = mybir.dt.float8e4
    f85 = mybir.dt.float8e5
    i32 = mybir.dt.int32
    u16 = mybir.dt.uint16
    u8 = mybir.dt.uint8
    AF = mybir.ActivationFunctionType
    OP = mybir.AluOpType
    DR = mybir.MatmulPerfMode.DoubleRow

    nc = bacc.Bacc(None, target_bir_lowering=False, debug=False)

    def param(name, shape, dtype=None):
        dtype = dtype or f32
        if inline_data is not None and name in inline_data:
            d = np.ascontiguousarray(np.asarray(inline_data[name]).reshape(shape))
            return nc.inline_tensor(d, name="il_" + name)
        return nc.declare_dram_parameter(name, shape, dtype, isOutput=False)

    # ---------------- dram parameters (packed: one DMA per SBUF tile) ----
    # x/mem are pre-split into d-pair tiles on the host; weights for each
    # attention block / the FFN are ONE contiguous blob each.
    xTb_h = param("xTb", [BPC, KP, 128, 2 * T], u16)   # bf16 x0 (residual)
    xT8_h = param("xT8", [BPC, KP, 128, 2 * T], u8)    # fp8 x0 (matmul)
    memT8_h = param("memT8", [BPC, KP, 128, 2 * S], u8)
    w = {}
    for p in ("sa", "ca"):
        # [wq kp0|wq kp1|wk kp0|wk kp1|wv kp0|wv kp1|wo] along free dim
        w[p + "_wpk"] = param(p + "_wpk", [128, 4 * (2 * A) + 2 * (2 * A2) + 2 * D], u8)
        w[p + "_bqo"] = param(p + "_bqo", [128, 2 * DT])   # bq cols 0:4, bo 4:8
        w[p + "_bv8"] = param(p + "_bv8", [128, 4], u8)    # bv k2 halves
    wffn_h = param("wffn", [128, 2 * (2 * FF) + (FF // 256) * (2 * D)], u8)
    b1row_h = param("b1row", [1, FF])
    b2_h = param("b2", [128, DT])
    sc2_h = param("sc2", [1, 2])                           # [norm_eps, bypass]
    out_h = nc.declare_dram_parameter("out", [BPC, DT, 128, T], f32, isOutput=True)
    tap_outs = {}

    # ---------------- inline constants (one blob) ----------------
    # cols 0:128 tri; 128:256 e4sel (rows 0:4); 256:258 onesd;
    # row0 258:386 ones1; row0 386:898 vpat (ones at 64h+32)
    cc = np.zeros((128, 1026), np.float32)
    cc[:, 0:128] = (np.arange(128)[:, None] <= np.arange(128)[None, :])
    for r in range(4):
        cc[r, 128 + 32 * r:128 + 32 * (r + 1)] = 1.0
    cc[:, 256:258] = 1.0
    cc[0, 258:386] = 1.0
    cc[0, 386 + 32:898:64] = 1.0
    # eselAV: broadcasts an av tile's den rows (32/96) to rb rows 0:64/64:128
    cc[32, 898:898 + 64] = 1.0
    cc[96, 898 + 64:898 + 128] = 1.0
    cc_h = nc.inline_tensor(cc, name="ccblob")

    with tile.TileContext(nc) as tc, ExitStack() as ctx:
        # pools
        wres = ctx.enter_context(tc.tile_pool(name="wres", bufs=1))
        consts = ctx.enter_context(tc.tile_pool(name="consts", bufs=1))
        xres = ctx.enter_context(tc.tile_pool(name="xres", bufs=12))
        x8p = ctx.enter_context(tc.tile_pool(name="x8p", bufs=8))
        memp = ctx.enter_context(tc.tile_pool(name="memp", bufs=4))
        qtp = ctx.enter_context(tc.tile_pool(name="qtp", bufs=5))
        ktp = ctx.enter_context(tc.tile_pool(name="ktp", bufs=5))
        vp = ctx.enter_context(tc.tile_pool(name="vp", bufs=6))
        expp = ctx.enter_context(tc.tile_pool(name="expp", bufs=6))
        avtp = ctx.enter_context(tc.tile_pool(name="avtp", bufs=2))
        rbp = ctx.enter_context(tc.tile_pool(name="rbp", bufs=6))
        smallp = ctx.enter_context(tc.tile_pool(name="smallp", bufs=6))
        ffa = ctx.enter_context(tc.tile_pool(name="ffa", bufs=6))
        outp = ctx.enter_context(tc.tile_pool(name="outp", bufs=2))
        # psum pools: big (2-bank slots) x2 + small (1-bank) x4 = 8 banks
        psb = ctx.enter_context(tc.tile_pool(name="psb", bufs=2, space="PSUM"))
        pss = ctx.enter_context(tc.tile_pool(name="pss", bufs=4, space="PSUM"))

        dma = nc.sync.dma_start
        ve, po, sc = nc.vector, nc.gpsimd, nc.scalar

        def tap(name, ap):
            if name not in taps or name in tap_outs:
                return
            shp = list(ap.shape)
            th = nc.declare_dram_parameter("tap_" + name, shp, f32, isOutput=True)
            tap_outs[name] = th
            dma(th[tuple(slice(0, n) for n in shp)], ap)

        def p2(ap2d, n=2):
            """[128, n*c] -> [128, n, c] free-dim split (k-tile pairs)."""
            return ap2d.rearrange("p (two c) -> p two c", two=n)

        # ---------------- load constants ----------------
        ccf = consts.tile([128, 1026], f32)
        dma(ccf[:], cc_h[:, :])
        tri_b = consts.tile([128, 128], bf)
        po.tensor_copy(tri_b[:], ccf[:, 0:128])
        esel0b = consts.tile([33, 128], bf)
        po.tensor_copy(esel0b[:], ccf[0:33, 898:1026])
        eselB = consts.tile([128, 128], bf)
        po.tensor_copy(eselB[64:97, :], ccf[64:97, 898:1026])
        onesd = consts.tile([128, 2], fr)
        po.tensor_copy(onesd[:], ccf[:, 256:258])
        ones1 = consts.tile([1, 128], fr)
        po.tensor_copy(ones1[:], ccf[0:1, 258:386])
        ones1f = ccf[0:1, 258:386]
        vpatb = consts.tile([1, 512], bf)
        po.tensor_copy(vpatb[:], ccf[0:1, 386:898])
        magicT = consts.tile([1, T], i32)
        po.memset(magicT[:], 0x5F3759DF)
        expb = consts.tile([128, 1], f32)
        po.memset(expb[:], -2.5)
        ones1b = consts.tile([1, 128], bf)
        po.memset(ones1b[:], 1.0)
        onesTfr = consts.tile([1, T], fr)
        po.tensor_scalar(onesTfr[:], ccf[0:1, 386:898], 0.0, 1.0, OP.mult, OP.add)
        m05 = consts.tile([128, 1], f32)
        po.memset(m05[:], -0.5)
        # scalars: epse = exp(norm_eps); bypass broadcast [128,1]
        nes = consts.tile([1, 2], f32)
        dma(nes[:], sc2_h[:, :])
        epse = consts.tile([1, 1], f32)
        sc.activation(epse[:], nes[:, 0:1], AF.Exp)
        bsps = pss.tile([128, 1], f32, tag="pss")
        nc.tensor.matmul(bsps[:], ones1f[:], nes[:, 1:2], start=True, stop=True)
        bspp = consts.tile([128, 1], f32)
        ve.tensor_scalar(bspp[:], bsps[:], 1.0, None, OP.mult)
        ombs = consts.tile([128, 1], f32)
        ve.tensor_scalar(ombs[:], bsps[:], -1.0, 1.0, OP.mult, OP.add)

        # ---------------- weight loads (sa first so SA starts asap) ------
        W = {}

        def load_x(b):
            """Returns (xbf pair tiles, x8 pair tiles) for batch b."""
            xb, x8 = [], []
            for j in range(KP):
                tb = xres.tile([128, 2 * T], bf, tag="x", name="x0")
                dma(tb[:], xTb_h[b, j].bitcast(bf))
                t8 = x8p.tile([128, 2 * T], f8, tag="x8", name="x08")
                dma(t8[:], xT8_h[b, j].bitcast(f8))
                xb.append(tb)
                x8.append(t8)
            return xb, x8

        def load_mem(b):
            m8 = []
            for j in range(KP):
                t8 = memp.tile([128, 2 * S], f8, tag="mem", name="mem8")
                dma(t8[:], memT8_h[b, j].bitcast(f8))
                m8.append(t8)
            return m8

        def load_attn_weights(p):
            # one blob DMA on the Activation hwdge queue (idle at startup)
            blob = wres.tile([128, 4 * (2 * A) + 2 * (2 * A2) + 2 * D], u8,
                             name=p + "_blob")
            sc.dma_start(blob[:], w[p + "_wpk"][:, :])
            bw = blob[:].bitcast(f8)
            W[p + "_wq8"] = [bw[:, 1024 * kp:1024 * (kp + 1)] for kp in range(KP)]
            W[p + "_wk8"] = [bw[:, 2048 + 1024 * kp:2048 + 1024 * (kp + 1)]
                             for kp in range(KP)]
            W[p + "_wv8"] = [bw[:, 4096 + 512 * kp:4096 + 512 * (kp + 1)]
                             for kp in range(KP)]
            W[p + "_wo8"] = bw[:, 5120:6144]

        def load_biases(p):
            bqo = wres.tile([128, 2 * DT], f32, name=p + "_bqo")
            dma(bqo[:], w[p + "_bqo"][:, :])
            W[p + "_bq"] = [bqo[:, m:m + 1] for m in range(DT)]
            W[p + "_bo"] = [bqo[:, DT + m:DT + m + 1] for m in range(DT)]
            bv = wres.tile([128, 4], f8, name=p + "_bv")
            dma(bv[:], w[p + "_bv8"][:, :].bitcast(f8))
            W[p + "_bv8"] = [bv[:, 2 * k2:2 * (k2 + 1)] for k2 in range(2)]

        pre = {}
        pre[0] = load_x(0)
        load_attn_weights("sa")
        load_biases("sa")
        # FFN weights (one blob on the ACT hwdge queue) + biases
        wffn = wres.tile([128, 2 * (2 * FF) + (FF // 256) * (2 * D)], u8,
                         name="wffn")
        sc.dma_start(wffn[:], wffn_h[:, :])
        wf8 = wffn[:].bitcast(f8)
        W1 = [wf8[:, 4096 * kp:4096 * (kp + 1)] for kp in range(KP)]
        W2 = [wf8[:, 8192 + 1024 * jp:8192 + 1024 * (jp + 1)]
              for jp in range(FF // 256)]
        b1rt = wres.tile([1, FF], fr, name="b1rt")
        dma(b1rt[:], b1row_h[:, :].bitcast(fr))
        b1row = [b1rt[0:1, 128 * j:128 * (j + 1)] for j in range(FF // 128)]
        b2t = wres.tile([128, DT], f32, name="b2t")
        dma(b2t[:], b2_h[:, :])
        b2pp = [b2t[:, m:m + 1] for m in range(DT)]
        pre_mem = {0: load_mem(0)}
        load_attn_weights("ca")
        load_biases("ca")
        pre[1] = load_x(1)
        pre_mem[1] = load_mem(1)

        # dbias[p][m] = bo + wo @ bv  (softmax rows sum to 1 -> v-bias folds
        # into the out-proj bias), [128,1] per d-tile; computed lazily so the
        # tiny matmuls never sit ahead of real work in the PE queue.
        dbias = {}

        def make_dbias(p):
            if p in dbias:
                return
            dbias[p] = []
            wo3 = p2(W[p + "_wo8"])
            for m in range(DT):
                ps = pss.tile([128, 2], f32, tag="pss")
                for k2 in range(2):
                    nc.tensor.matmul(ps[:], wo3[:, k2, 128 * m:128 * (m + 1)],
                                     W[p + "_bv8"][k2], start=(k2 == 0), stop=(k2 == 1))
                t_ = wres.tile([128, 1], f32, name=f"{p}_dbias_{m}")
                ve.tensor_scalar(t_[:], ps[:, 0:1], W[p + "_bo"][m], None, OP.add)
                dbias[p].append(t_)

        # ------------------------------------------------------------------
        def kv_proj(p, kv8, kvlen):
            """K/V projections depend only on the kv input. Returns
            (KT, VP, units): each unit is a closure emitting one tile's
            matmuls+copy -- popped as PE filler inside a neighboring
            attention core's exp-wait bubbles."""
            ST = kvlen // 128
            NPAIR = ST // 2
            wk = [p2(t) for t in W[p + "_wk8"]]
            wv = [p2(t) for t in W[p + "_wv8"]]
            kv3 = [p2(t[:]) for t in kv8]
            QT, KT = [], [None] * DT
            VP = [None] * NPAIR
            units = []

            # K bias is dropped: it adds a per-t constant to every score row,
            # which cancels exactly in the softmax normalization. For CA the
            # two 512-wide s-chunks of each m share one 2-bank psum and one
            # [128,1024] copy.
            def k_unit(m, scc):
                if scc == 0:
                    KT[m] = ktp.tile([128, kvlen], bf, tag="kt", name="kt")
                ps = pss.tile([128, 512], f32, tag="pss")
                for kp in range(KP):
                    nc.tensor.matmul(ps[:], wk[kp][:, :, 128 * m:128 * (m + 1)],
                                     kv3[kp][:, :, 512 * scc:512 * (scc + 1)]
                                     if kvlen > 512 else kv3[kp],
                                     start=(kp == 0), stop=(kp == KP - 1),
                                     perf_mode=DR)
                ve.tensor_copy(KT[m][:, 512 * scc:512 * (scc + 1)], ps[:])

            # V tiles [128, 2, 8, 64] fp8: head slot h at cols [64h, 64h+33):
            # 32 value dims + ones col (seeded by a rank-1 matmul which also
            # zeros the slack so the psum->sbuf copy is contiguous)
            def v_unit(st):
                pr, sti = st // 2, st % 2
                if sti == 0:
                    VP[pr] = vp.tile([128, 1024], f8, tag="vt", name="vt")
                vt4 = VP[pr][:].rearrange("p (two h c) -> p two h c", two=2, c=64)
                ps = pss.tile([128, A2], f32, tag="pss")
                for kp in range(KP):
                    nc.tensor.matmul(ps[:], kv3[kp][:, :, 128 * st:128 * (st + 1)],
                                     wv[kp], start=(kp == 0), stop=(kp == KP - 1),
                                     perf_mode=DR)
                ve.tensor_copy(vt4[:, sti, :, 0:32],
                               ps[:].rearrange("p (h c) -> p h c", c=32))
                ve.memset(vt4[:, sti, :, 32:33], 1.0)

            for m in range(DT):
                for scc in range(kvlen // 512):
                    units.append(lambda m=m, scc=scc: k_unit(m, scc))
            for st in range(ST):
                units.append(lambda st=st: v_unit(st))
            return KT, VP, units

        def attention(p, xq8, kvpack, kvlen, causal, resid_bf, want_x8,
                      fillers=None):
            """xq8: fp8 d-pair tiles [128,2,T]; kvpack from kv_proj."""
            ST = kvlen // 128
            NPAIR = ST // 2
            KT, VP, _ = kvpack
            fillers = list(fillers or [])
            wq = [p2(t) for t in W[p + "_wq8"]]
            wo3 = p2(W[p + "_wo8"])
            bq = W[p + "_bq"]
            xq3 = [p2(t[:]) for t in xq8]

            # --- Q projection (DoubleRow over d-pairs) ---
            QT = []
            for m in range(DT):
                ps = pss.tile([128, T], f32, tag="pss")
                for kp in range(KP):
                    nc.tensor.matmul(ps[:], wq[kp][:, :, 128 * m:128 * (m + 1)],
                                     xq3[kp], start=(kp == 0), stop=(kp == KP - 1),
                                     perf_mode=DR)
                q = qtp.tile([128, T], bf, tag="q")
                sc.activation(q[:], ps[:], AF.Identity, bias=bq[m])
                tap(f"{p}_QT{m}", q[:])
                QT.append(q)

            # --- scores -> exp(fp8) -> DR AV, per (hp, pair, hl) ---
            av_pair = {}
            av_sb = {}
            rb_sb = {}
            avt8 = avtp.tile([128, 2 * T], f8, tag="avt")
            avt3 = p2(avt8[:])
            xo_bf, xo_8 = [], []
            def emit_av(job):
                hp, pr, hl, ex3, t0 = job
                h = 2 * hp + hl
                nc.tensor.matmul(
                    av_pair[hp][hl][0:33, t0:T],
                    p2(VP[pr][:])[:, :, 64 * h:64 * h + 33],
                    ex3, start=(pr == 0), stop=(pr == NPAIR - 1),
                    perf_mode=DR)

            LAG = 2
            for hp in range(4):
                half = hp // 2
                av0 = pss.tile([128, T], f32, tag="pss", name="av0")
                av1 = pss.tile([128, T], f32, tag="pss", name="av1")
                av_pair[hp] = (av0, av1)
                pending = []
                for pr in range(NPAIR):
                    if causal:
                        w0, w1 = T - 256 * pr, T - 256 * pr - 128
                    else:
                        w0 = w1 = 512
                    t0 = T - w0
                    for hl in range(2):
                        h = 2 * hp + hl
                        sl = slice(64 * hl, 64 * (hl + 1))
                        scp = psb.tile([128, w0 + w1], f32, tag="psb", name="scps")
                        nc.tensor.matmul(
                            scp[:, 0:w0],
                            KT[hp][sl, 256 * pr:256 * pr + 128],
                            QT[hp][sl, t0:T], start=True, stop=True)
                        nc.tensor.matmul(
                            scp[:, w0:w0 + w1],
                            KT[hp][sl, 256 * pr + 128:256 * pr + 256],
                            QT[hp][sl, T - w1:T], start=True, stop=True)
                        ex = expp.tile([128, 2 * w0], f85, tag="exp")
                        ex3 = p2(ex[:])
                        if w0 == w1:
                            sc.activation(ex[:, 0:2 * w0], scp[:, 0:w0 + w1],
                                          AF.Exp, bias=expb[:])
                        else:
                            sc.activation(ex3[:, 0, :], scp[:, 0:w0], AF.Exp,
                                          bias=expb[:])
                            sc.activation(ex3[:, 1, w0 - w1:w0], scp[:, w0:w0 + w1],
                                          AF.Exp, bias=expb[:])
                        if causal:
                            ve.tensor_tensor(ex3[:, 0, 0:128], ex3[:, 0, 0:128],
                                             tri_b[:], OP.mult)
                            ve.memset(ex3[:, 1, 0:128], 0.0)
                            ve.tensor_tensor(ex3[:, 1, 128:256], ex3[:, 1, 128:256],
                                             tri_b[:], OP.mult)
                        tap(f"{p}_ex{h}_{pr}", ex[:])
                        pending.append((hp, pr, hl, ex3, t0))
                        if fillers:
                            fillers.pop(0)()
                        if len(pending) > LAG:
                            emit_av(pending.pop(0))
                for job in pending:
                    emit_av(job)
                # av psum -> sbuf (frees the bank early); denominators are
                # pulled out of rows 32/96 by the esel matmuls, inverted once,
                # and the per-head normalization runs on Pool (SBUF-only)
                avs = rbp.tile([128, T], bf, tag="avsb", name="avsb")
                ve.tensor_copy(avs[0:33, :], av_pair[hp][0][0:33, :])
                ve.tensor_copy(avs[64:97, :], av_pair[hp][1][0:33, :])
                av_sb[hp] = avs
                rb_ps = pss.tile([128, T], f32, tag="pss", name="rbps")
                nc.tensor.matmul(rb_ps[:], esel0b[:], avs[0:33, :],
                                 start=True, stop=False)
                nc.tensor.matmul(rb_ps[:], eselB[64:97, :], avs[64:97, :],
                                 start=False, stop=True)
                rbs = rbp.tile([128, T], f32, tag="rb", name="rbs")
                ve.reciprocal_approx_fast(rbs[:], rb_ps[:])
                rb_sb[hp] = rbs
                if hp % 2 == 1:
                    for hp2 in (hp - 1, hp):
                        for hl in range(2):
                            r = 2 * hl + (hp2 % 2)
                            po.tensor_mul(
                                avt8[32 * r:32 * r + 32, half * T:(half + 1) * T],
                                av_sb[hp2][64 * hl:64 * hl + 32, :],
                                rb_sb[hp2][64 * hl:64 * hl + 32, :])
            tap(f"{p}_avt", avt8[:])
            for f_ in fillers:
                f_()

            # out-proj (DR over the two A2 k-tiles) + bias + residual
            make_dbias(p)
            for m in range(DT):
                j, i = m // 2, m % 2
                if i == 0:
                    xn = xres.tile([128, 2 * T], bf, tag="x")
                    xo_bf.append(xn)
                    if want_x8:
                        x8n = x8p.tile([128, 2 * T], f8, tag="x8")
                        xo_8.append(x8n)
                ps = pss.tile([128, T], f32, tag="pss")
                nc.tensor.matmul(ps[:], wo3[:, :, 128 * m:128 * (m + 1)], avt3,
                                 start=True, stop=True, perf_mode=DR)
                ve.scalar_tensor_tensor(p2(xo_bf[j][:])[:, i, :], ps[:],
                                        dbias[p][m][:], p2(resid_bf[j][:])[:, i, :],
                                        OP.add, OP.add)
                if want_x8:
                    po.tensor_copy(p2(xo_8[j][:])[:, i, :], p2(xo_bf[j][:])[:, i, :])
                tap(f"{p}_x{m}", p2(xo_bf[j][:])[:, i, :])
            return xo_bf, (xo_8 if want_x8 else None)

        # ------------------------------------------------------------------
        def ffn(xbf, x8):
            # b1 is seeded into the h psum with a rank-1 fp32r matmul, so the
            # psum holds g = h + b1 directly: one [128,1024] tanh (const bias),
            # one Pool tensor_scalar, one DVE tensor_tensor per j-PAIR.
            x3 = [p2(t[:]) for t in x8]
            w1 = [p2(t) for t in W1]
            acc = [pss.tile([128, T], f32, name="ffacc", tag="pss") for _ in range(DT)]
            def emit_w2(jp, hs3):
                w2j = p2(W2[jp])
                for m in range(DT):
                    nc.tensor.matmul(acc[m][:], w2j[:, :, 128 * m:128 * (m + 1)],
                                     hs3, start=(jp == 0), stop=(jp == FF // 256 - 1),
                                     perf_mode=DR)

            prev = None
            for jp in range(FF // 256):
                hswp = ffa.tile([128, 1024], f8, tag="hsw")
                hs3 = p2(hswp[:])
                hp2 = psb.tile([128, 1024], f32, tag="psb")
                h3 = p2(hp2[:])
                for i2 in range(2):
                    j = 2 * jp + i2
                    nc.tensor.matmul(h3[:, i2, :], b1row[j], onesTfr[:],
                                     start=True, stop=False)
                    for kp in range(KP):
                        nc.tensor.matmul(h3[:, i2, :],
                                         w1[kp][:, :, 128 * j:128 * (j + 1)],
                                         x3[kp], start=False, stop=(kp == KP - 1),
                                         perf_mode=DR)
                th = ffa.tile([128, 1024], bf, tag="th")
                sc.activation(th[:], hp2[:], AF.Tanh, bias=m05[:], scale=0.5)
                thp = ffa.tile([128, 1024], bf, tag="thp")
                po.tensor_scalar(thp[:], th[:], 0.5, 0.5, OP.mult, OP.add)
                # hsw = g * sigmoid(g - 1), g = h + b1 (already in psum)
                ve.tensor_tensor(hswp[:], hp2[:], thp[:], OP.mult)
                if prev is not None:
                    emit_w2(*prev)
                prev = (jp, hs3)
            emit_w2(*prev)
            xo = []
            for m in range(DT):
                j, i = m // 2, m % 2
                if i == 0:
                    xn = xres.tile([128, 2 * T], bf, tag="x")
                    xo.append(xn)
                ve.scalar_tensor_tensor(
                    p2(xo[j][:])[:, i, :], acc[m][:],
                    b2pp[m], p2(xbf[j][:])[:, i, :], OP.add, OP.add)
            return xo

        # ------------------------------------------------------------------
        def norm_bypass(b, x3, x0):
            # v = mean(x^2) + eps; rsqrt via int bit-trick + 2 Newton steps
            vps = pss.tile([2, T], f32, tag="pss")
            for m in range(DT):
                j, i = m // 2, m % 2
                sq = smallp.tile([128, T], fr, tag="small")
                po.tensor_tensor(sq[:], p2(x3[j][:])[:, i, :], p2(x3[j][:])[:, i, :],
                                 OP.mult)
                nc.tensor.matmul(vps[:], onesd[:], sq[:], start=(m == 0),
                                 stop=(m == DT - 1))
            vv = smallp.tile([1, T], f32, tag="small")
            ve.tensor_scalar(vv[:], vps[0:1, :], 1.0 / 512.0, epse[:], OP.mult, OP.add)
            sh = smallp.tile([1, T], i32, tag="small")
            ve.tensor_scalar(sh[:], vv[:].bitcast(i32), 1, None,
                             OP.logical_shift_right)
            r0 = smallp.tile([1, T], f32, tag="small")
            ve.tensor_tensor(r0[:].bitcast(i32), magicT[:], sh[:], OP.subtract)
            hv = smallp.tile([1, T], f32, tag="small")
            po.tensor_scalar(hv[:], vv[:], 0.5, None, OP.mult)
            t1 = smallp.tile([1, T], f32, tag="small")
            t2 = smallp.tile([1, T], f32, tag="small")
            rr = smallp.tile([1, T], fr, tag="small")
            po.tensor_mul(t1[:], hv[:], r0[:])
            po.tensor_mul(t2[:], t1[:], r0[:])
            po.tensor_scalar(t2[:], t2[:], -1.0, 1.5, OP.mult, OP.add)
            po.tensor_mul(rr[:], r0[:], t2[:])
            sqb = pss.tile([128, T], f32, tag="pss")
            nc.tensor.matmul(sqb[:], ones1[:], rr[:], start=True, stop=True)
            ot = outp.tile([128, DT * T], f32, tag="out")
            for m in range(DT):
                j, i = m // 2, m % 2
                u = smallp.tile([128, T], f32, tag="small")
                ve.scalar_tensor_tensor(u[:], p2(x3[j][:])[:, i, :], bspp[:],
                                        sqb[:], OP.mult, OP.mult)
                ve.scalar_tensor_tensor(ot[:, T * m:T * (m + 1)],
                                        p2(x0[j][:])[:, i, :], ombs[:], u[:],
                                        OP.mult, OP.add)
            dma(out_h[b].rearrange("m p t -> p m t"),
                ot[:].rearrange("p (m t) -> p m t", m=DT))

        # ------------------------------------------------------------------
        # Phase-staggered emission: SA0 CA0 SA1 FFN0 CA1 norm0 FFN1 norm1.
        # Exp-heavy attention segments interleave with matmul-heavy FFN/norm
        # segments in every engine queue, and each batch's inputs prefetch an
        # iteration ahead of its output DMAs in the (in-order) SP DMA queue.
        def dump_x(b, xbf):
            ot = outp.tile([128, DT * T], f32, tag="out")
            for m in range(DT):
                j, i = m // 2, m % 2
                po.tensor_scalar(ot[:, T * m:T * (m + 1)], p2(xbf[j][:])[:, i, :],
                                 1.0, None, OP.mult)
            dma(out_h[b].rearrange("m p t -> p m t"),
                ot[:].rearrange("p (m t) -> p m t", m=DT))

        for it in range(unroll):
            x0bf, x08, mem8, x1bf, x18, x2bf, x28, x3bf = ({} for _ in range(8))
            for b in range(BPC):
                x0bf[b], x08[b] = pre[b]
                mem8[b] = pre_mem[b]
            if stop_after == "dma":
                for b in range(BPC):
                    dump_x(b, x0bf[b])
                continue
            sa_kv0 = kv_proj("sa", x08[0], T)
            for u in sa_kv0[2]:
                u()
            x1bf[0], x18[0] = attention("sa", x08[0], sa_kv0, T, True, x0bf[0],
                                        True)
            if stop_after == "sa":
                sa_kv1 = kv_proj("sa", x08[1], T)
                for u in sa_kv1[2]:
                    u()
                x1bf[1], _ = attention("sa", x08[1], sa_kv1, T, True, x0bf[1], False)
                dump_x(0, x1bf[0])
                dump_x(1, x1bf[1])
                continue
            ca_kv0 = kv_proj("ca", mem8[0], S)
            for u in ca_kv0[2]:
                u()
            x2bf[0], x28[0] = attention("ca", x18[0], ca_kv0, S, False, x1bf[0],
                                        True)
            if stop_after == "ca":
                dump_x(0, x2bf[0])
                dump_x(1, x1bf[1] if 1 in x1bf else x0bf[1])
                continue
            x3bf[0] = ffn(x2bf[0], x28[0])
            if stop_after == "ffn":
                dump_x(0, x3bf[0])
                dump_x(1, x0bf[1])
                continue
            sa_kv1 = kv_proj("sa", x08[1], T)
            for u in sa_kv1[2]:
                u()
            x1bf[1], x18[1] = attention("sa", x08[1], sa_kv1, T, True, x0bf[1],
                                        True)
            ca_kv1 = kv_proj("ca", mem8[1], S)
            for u in ca_kv1[2]:
                u()
            x2bf[1], x28[1] = attention("ca", x18[1], ca_kv1, S, False, x1bf[1],
                                        True)
            norm_bypass(0, x3bf[0], x0bf[0])
            x3bf[1] = ffn(x2bf[1], x28[1])
            if it + 1 < unroll:
                for b in range(BPC):
                    pre[b] = load_x(b)
                    pre_mem[b] = load_mem(b)
            norm_bypass(1, x3bf[1], x0bf[1])

    nc.compile()
    return nc


# ----------------------------------------------------------------------------
# host-side runner (cached jit via PJRT / axon)
# ----------------------------------------------------------------------------

class _Runner:
    def __init__(self, nc, n_cores=NCORES):
        import jax
        import numpy as _np
        from jax.sharding import Mesh, PartitionSpec
        from jax.experimental.shard_map import shard_map
        import concourse.mybir as mybir
        from concourse.bass2jax import (_bass_exec_p, install_neuronx_cc_hook,
                                        partition_id_tensor)
        install_neuronx_cc_hook()
        self.jax = jax
        self.n_cores = n_cores
        in_names, out_names, out_avals, zero_outs = [], [], [], []
        for alloc in nc.m.functions[0].allocations:
            if not isinstance(alloc, mybir.MemoryLocationSet):
                continue
            name = alloc.memorylocations[0].name
            if alloc.kind == "ExternalInput":
                if nc.partition_id_tensor is not None and name == nc.partition_id_tensor.name:
                    continue
                in_names.append(name)
            elif alloc.kind == "ExternalOutput":
                out_names.append(name)
                shape = tuple(alloc.tensor_shape)
                dtype = mybir.dt.np(alloc.dtype)
                out_avals.append(jax.core.ShapedArray(shape, dtype))
                zero_outs.append(_np.zeros(shape, dtype))
        self.in_names, self.out_names = in_names, out_names
        self.out_avals, self.zero_outs = out_avals, zero_outs
        part_name = nc.partition_id_tensor.name if nc.partition_id_tensor else None
        all_in = in_names + out_names + ([part_name] if part_name else [])

        def _body(*args):
            operands = list(args)
            if part_name is not None:
                operands.append(partition_id_tensor())
            outs = _bass_exec_p.bind(
                *operands, out_avals=tuple(out_avals), in_names=tuple(all_in),
                out_names=tuple(out_names), lowering_input_output_aliases=(),
                sim_require_finite=True, sim_require_nnan=True, nc=nc)
            return tuple(outs)

        devices = jax.devices()[:n_cores]
        mesh = Mesh(np.asarray(devices), ("core",))
        n_params = len(in_names)
        self.sharded = jax.jit(
            shard_map(_body, mesh=mesh,
                      in_specs=(PartitionSpec("core"),) * (n_params + len(out_names)),
                      out_specs=(PartitionSpec("core"),) * len(out_names),
                      check_rep=False),
            keep_unused=True)

    def put(self, in_maps):
        jax = self.jax
        per_core = [[np.asarray(m[nm]) for nm in self.in_names] for m in in_maps]
        args = [np.concatenate([per_core[c][i] for c in range(self.n_cores)], axis=0)
                for i in range(len(self.in_names))]
        args += [np.zeros((self.n_cores * z.shape[0], *z.shape[1:]), z.dtype)
                 for z in self.zero_outs]
        self._dev_args = jax.block_until_ready([jax.device_put(a) for a in args])
        return self._dev_args

    def run(self, in_maps=None):
        jax = self.jax
        if in_maps is not None:
            self.put(in_maps)
        out_arrs = jax.block_until_ready(self.sharded(*self._dev_args))
        return [
            {nm: np.asarray(out_arrs[i]).reshape(self.n_cores, *self.out_avals[i].shape)[c]
             for i, nm in enumerate(self.out_names)}
            for c in range(self.n_cores)
        ]


def _numpy_reference(tgt, memory, tgt_mask, memory_mask, **kw):
    def lin(x, wm, bb):
        return x @ wm.T + bb

    def mha(xq, xkv, wq, bq, wk, bk, wv, bv, wo, bo, mask):
        b_, t_, _ = xq.shape
        s_ = xkv.shape[1]
        q = lin(xq, wq, bq).reshape(b_, t_, NH, HD)
        k = lin(xkv, wk, bk).reshape(b_, s_, NH, HD)
        v = lin(xkv, wv, bv).reshape(b_, s_, NH, HD2)
        scr = np.einsum('bthd,bshd->bhts', q, k)
        scr = np.where(mask[:, None, :, :], -np.inf, scr)
        scr = scr - scr.max(axis=-1, keepdims=True)
        e = np.exp(scr)
        at = e / e.sum(axis=-1, keepdims=True)
        o = np.einsum('bhts,bshd->bthd', at, v).reshape(b_, t_, A2)
        return lin(o, wo, bo)

    x = tgt + mha(tgt, tgt, kw['sa_wq'], kw['sa_bq'], kw['sa_wk'], kw['sa_bk'],
                  kw['sa_wv'], kw['sa_bv'], kw['sa_wo'], kw['sa_bo'], tgt_mask)
    x = x + mha(x, memory, kw['ca_wq'], kw['ca_bq'], kw['ca_wk'], kw['ca_bk'],
                kw['ca_wv'], kw['ca_bv'], kw['ca_wo'], kw['ca_bo'], memory_mask)
    h = lin(x, kw['ff_w1'], kw['ff_b1'])
    h = h / (1.0 + np.exp(1.0 - h))
    x = x + lin(h, kw['ff_w2'], kw['ff_b2'])
    y = x / np.sqrt((x * x).mean(-1, keepdims=True) + np.exp(kw['norm_eps']))
    return tgt + (y - tgt) * kw['bypass_scale']


def _masks_standard(tgt_mask, memory_mask):
    causal = ~np.tril(np.ones((T, T), bool))
    return (np.array_equal(np.asarray(tgt_mask),
                           np.broadcast_to(causal, (B, T, T))) and
            not np.asarray(memory_mask).any())


def _pack_pairs(wT):
    """[D, X] -> [KP, 128, 2, X]: row-tile pairs for DoubleRow lhsT."""
    Drows, X = wT.shape
    return np.ascontiguousarray(
        wT.reshape(Drows // 256, 2, 128, X).transpose(0, 2, 1, 3))


def _pair_cols(xT):
    """[D, N] -> [KP, 128, 2*N] d-pair tiles (halves side by side)."""
    N = xT.shape[1]
    return np.ascontiguousarray(
        xT.reshape(KP, 2, 128, N).transpose(0, 2, 1, 3)).reshape(KP, 128, 2 * N)


def make_in_maps(inputs):
    f = np.float32
    import ml_dtypes
    e4m3 = ml_dtypes.float8_e4m3
    bfv = lambda a: np.ascontiguousarray(
        np.asarray(a, f).astype(ml_dtypes.bfloat16)).view(np.uint16)
    f8v = lambda a: np.ascontiguousarray(np.asarray(a, f).astype(e4m3)).view(np.uint8)

    def pk2(wT, X):  # [n,128,2,X] -> [128, n*2X] fp8 blob columns
        return np.ascontiguousarray(
            f8v(_pack_pairs(wT)).transpose(1, 0, 2, 3)).reshape(128, -1)

    shared = {
        "b1row": np.asarray(inputs["ff_b1"], f).reshape(1, FF),
        "b2": np.asarray(inputs["ff_b2"], f).reshape(DT, 128).T.copy(),
        "sc2": np.asarray([np.float32(inputs["norm_eps"]),
                           np.float32(inputs["bypass_scale"])], f).reshape(1, 2),
    }
    shared["wffn"] = np.concatenate(
        [pk2(np.asarray(inputs["ff_w1"], f).T, FF),
         pk2(np.asarray(inputs["ff_w2"], f).T, D)], axis=1)
    for p in ("sa", "ca"):
        wq = pk2(np.asarray(inputs[p + "_wq"], f).T, A)
        wk = pk2(np.asarray(inputs[p + "_wk"], f).T, A)
        wv = pk2(np.asarray(inputs[p + "_wv"], f).T, A2)
        woT = np.asarray(inputs[p + "_wo"], f).T.reshape(NH, HD2, D)
        perm = [0, 2, 1, 3, 4, 6, 5, 7]
        wo = f8v(woT[perm].reshape(2, 128, D).transpose(1, 0, 2)).reshape(128, 2 * D)
        shared[p + "_wpk"] = np.concatenate([wq, wk, wv, wo], axis=1)
        bq = np.asarray(inputs[p + "_bq"], f).reshape(DT, 128).T
        bo = np.asarray(inputs[p + "_bo"], f).reshape(DT, 128).T
        shared[p + "_bqo"] = np.concatenate([bq, bo], axis=1).copy()
        bv = np.asarray(inputs[p + "_bv"], f).reshape(NH, HD2)[[0, 2, 1, 3, 4, 6, 5, 7]]
        bv2 = np.stack([bv.reshape(A2), bv.reshape(A2)], axis=1)
        shared[p + "_bv8"] = f8v(bv2.reshape(2, 128, 2).transpose(1, 0, 2).reshape(128, 4))
    tgt = np.asarray(inputs["tgt"], f)
    memory = np.asarray(inputs["memory"], f)
    in_maps = []
    for c in range(NCORES):
        sl = slice(BPC * c, BPC * (c + 1))
        m = dict(shared)
        xT = tgt[sl].transpose(0, 2, 1)
        m["xTb"] = np.stack([bfv(_pair_cols(x)) for x in xT])
        m["xT8"] = np.stack([f8v(_pair_cols(x)) for x in xT])
        m["memT8"] = np.stack([f8v(_pair_cols(x))
                               for x in memory[sl].transpose(0, 2, 1)])
        in_maps.append(m)
    return in_maps


def kernel(**inputs):
    global _RUNNER
    if not _masks_standard(inputs["tgt_mask"], inputs["memory_mask"]):
        return _numpy_reference(**{k: np.asarray(v, np.float64) if np.asarray(v).dtype != bool else np.asarray(v)
                                   for k, v in inputs.items()}).astype(np.float32)
    if _RUNNER is None:
        _RUNNER = _Runner(build_nc())
    res = _RUNNER.run(make_in_maps(inputs))
    out = np.concatenate([r["out"] for r in res], axis=0)  # [B, DT, 128, T]
    return np.ascontiguousarray(out.transpose(0, 3, 1, 2).reshape(B, T, D))
